# revision 1
# baseline (speedup 1.0000x reference)
"""KNN-graph (DGM euclidean) kernel for Trainium2, sharded over 8 NeuronCores.

Problem: x [1, 8192, 256] f32 -> (x, edges [2, 81920] i32, logprobs [1, 8192, 10] f32)
(k=10 nearest neighbors under squared euclidean distance, scaled by
exp(clip(temperature, -5, 5)); edges/logprobs replicate the torch DGM layout
quirks of the reference, including the k*n vs n*k pairing mismatch in x1/x2.)

Sharding: rows are split 1024-per-core (8 cores); every core receives the
full x^T for the rhs of its [1024, 8192] distance-tile matmul (host-side
"all-gather").

Per core:
  phase 1 (top-k): neg_rank[i,j] = <x_i,x_j> - sq_j/2 is monotone-decreasing
    in ||x_i - x_j||^2, so top-10-smallest-distance == top-10-largest
    neg_rank.  The gram matrix is computed in fp16 hi/lo split form
    (x = h + l exactly, g = h.h + h.l + l.h + l.l accumulated in fp32 PSUM
    -> fp32-grade precision at the PE's full fp16 rate; plain fp32 matmuls
    run at 1/4 rate).  The -sq_j/2 term rides in as one K=2 matmul against a
    constant ones lhsT.  Distances land in [128, 512] PSUM chunks; the DVE's
    max (top-8 per partition) + max_index (their within-chunk indices) reduce
    each chunk; a second max/max_index/match_replace/max/max_index pass over
    the 128 per-row candidates yields the global top-16 values + candidate
    positions.  Host maps (position -> chunk, within-chunk index) to absolute
    neighbor indices.  Exact-duplicate values are handled the same way
    jax.lax.top_k does (ascending index for ties): max_index assigns
    successive occurrences to successive ranks, and ranks 9-16 are searched
    in the match_replace'd candidate array so values spanning the 8/9
    boundary resolve to the correct occurrence.
  phase 2 (logprobs): the reference pairs x1[m]=x[flat[m]] (order k*n) with
    x2[m]=x[m//k] (order n*k), i.e. logprobs[i,c] = -scale*||x[A[i,c]]-x[i]||^2
    with A = flat.reshape(n, k), flat = idx.T.flatten().  A depends on the
    full idx, so it runs as a second launch: indirect-DMA row gather of
    x[A[i,c]], subtract broadcast x_i, Square on the scalar engine, free-dim
    reduce, scale by -scale.

Everything else (edge assembly, index arithmetic) is cheap integer
bookkeeping done on host, replicating the reference expressions verbatim.
"""

import numpy as np
import orjson

import concourse.bass as bass
import concourse.mybir as mybir
import concourse.tile as tile

# ---------------------------------------------------------------------------
# Toolchain workaround: the walrus build in this container rejects
# instructions carrying more than one sem-wait ("Too many sync wait
# commands", CoreV3GenImpl.cpp setupSyncWait).  TileContext emits multi-wait
# instructions (notably its tail drain, matmuls and DMA triggers).  Splitting
# is safe: extra waits are hoisted onto Drain instructions inserted
# immediately before the owner on the same engine queue — queues execute
# in-order, so all waits still complete before the instruction issues.
# ---------------------------------------------------------------------------
_MAX_WAITS = 1
_patched = False


def _split_bir_waits(bir_json: bytes) -> bytes:
    bir = orjson.loads(bir_json)
    n_fixed = 0
    for fn in bir.get("functions", []):
        for bb in fn.get("blocks", []):
            out = []
            for ins in bb.get("instructions", []):
                si = ins.get("sync_info")
                waits = si.get("on_wait") if si else None
                if waits and len(waits) > _MAX_WAITS:
                    extra = waits[_MAX_WAITS:]
                    si["on_wait"] = waits[:_MAX_WAITS]
                    for i in range(0, len(extra), _MAX_WAITS):
                        out.append(
                            {
                                "name": f"{ins['name']}-wsplit{i}",
                                "opcode": "Drain",
                                "engine": ins["engine"],
                                "ins": [],
                                "outs": [],
                                "debug": ins.get("debug", 0),
                                "sync_info": {
                                    "on_wait": extra[i : i + _MAX_WAITS],
                                    "on_update": [],
                                },
                            }
                        )
                        n_fixed += 1
                out.append(ins)
            bb["instructions"] = out
    if n_fixed:
        return orjson.dumps(bir)
    return bir_json


def install_patch():
    global _patched
    if _patched:
        return
    _patched = True
    from concourse import bass_utils, bass2jax

    orig = bass_utils.compile_bir_kernel

    def patched_compile_bir_kernel(bir_json, tmpdir, neff_name="file.neff"):
        return orig(_split_bir_waits(bytes(bir_json)), tmpdir, neff_name)

    bass_utils.compile_bir_kernel = patched_compile_bir_kernel
    bass2jax.compile_bir_kernel = patched_compile_bir_kernel


install_patch()

from concourse.bass_utils import run_bass_kernel_spmd  # noqa: E402

f32 = mybir.dt.float32
f16 = mybir.dt.float16
u32 = mybir.dt.uint32
i32 = mybir.dt.int32

N, D, KK = 8192, 256, 10
CORES = 8
ROWS = N // CORES  # 1024 rows per core
P = 128
BLKS = ROWS // P  # 8 row-blocks per core
CH = 512  # distance-chunk width (one PSUM bank)
NB = N // CH  # 16 chunks across the 8192 columns
CPC = 8 * NB  # candidates per row (128)
NEG_BIG = -3.0e38


def build_phase1() -> bass.Bass:
    nc = bass.Bass("TRN2", target_bir_lowering=False, debug=False)
    xth_d = nc.dram_tensor("xth", [D, N], f16, kind="ExternalInput")
    xtl_d = nc.dram_tensor("xtl", [D, N], f16, kind="ExternalInput")
    lhh_d = nc.dram_tensor("lhh", [D, ROWS], f16, kind="ExternalInput")
    lhl_d = nc.dram_tensor("lhl", [D, ROWS], f16, kind="ExternalInput")
    msq_d = nc.dram_tensor("msq", [2, N], f16, kind="ExternalInput")
    one_d = nc.dram_tensor("one", [2, P], f16, kind="ExternalInput")
    pos_d = nc.dram_tensor("pos", [BLKS, P, 16], u32, kind="ExternalOutput")
    rel_d = nc.dram_tensor("rel", [BLKS, P, CPC], u32, kind="ExternalOutput")
    val_d = nc.dram_tensor("val", [BLKS, P, 16], f32, kind="ExternalOutput")

    with tile.TileContext(nc) as tc:
        with (
            tc.tile_pool(name="const", bufs=1) as const,
            tc.tile_pool(name="work", bufs=3) as work,
            tc.tile_pool(name="psp", bufs=8, space="PSUM") as psum,
        ):
            xth0 = const.tile([P, N], f16)
            xth1 = const.tile([P, N], f16)
            xtl0 = const.tile([P, N], f16)
            xtl1 = const.tile([P, N], f16)
            nc.sync.dma_start(xth0[:], xth_d.ap()[0:P, :])
            nc.sync.dma_start(xth1[:], xth_d.ap()[P : 2 * P, :])
            nc.sync.dma_start(xtl0[:], xtl_d.ap()[0:P, :])
            nc.sync.dma_start(xtl1[:], xtl_d.ap()[P : 2 * P, :])
            lhh0 = const.tile([P, ROWS], f16)
            lhh1 = const.tile([P, ROWS], f16)
            lhl0 = const.tile([P, ROWS], f16)
            lhl1 = const.tile([P, ROWS], f16)
            nc.sync.dma_start(lhh0[:], lhh_d.ap()[0:P, :])
            nc.sync.dma_start(lhh1[:], lhh_d.ap()[P : 2 * P, :])
            nc.sync.dma_start(lhl0[:], lhl_d.ap()[0:P, :])
            nc.sync.dma_start(lhl1[:], lhl_d.ap()[P : 2 * P, :])
            msq_t = const.tile([2, N], f16)
            one_t = const.tile([2, P], f16)
            nc.sync.dma_start(msq_t[:], msq_d.ap())
            nc.sync.dma_start(one_t[:], one_d.ap())

            for blk in range(BLKS):
                cand = work.tile([P, CPC], f32, tag="cand")
                relx = work.tile([P, CPC], u32, tag="relx")
                ms = slice(blk * P, (blk + 1) * P)
                for nb in range(NB):
                    cs = slice(nb * CH, (nb + 1) * CH)
                    ps = psum.tile([P, CH], f32, tag="ps")
                    # g = h.h + h.l + l.h + l.l  (exact fp32-grade gram), plus
                    # the -sq_j/2 row via a K=2 ones matmul.
                    nc.tensor.matmul(ps[:], lhh0[:, ms], xth0[:, cs], start=True, stop=False)
                    nc.tensor.matmul(ps[:], lhh1[:, ms], xth1[:, cs], start=False, stop=False)
                    nc.tensor.matmul(ps[:], lhh0[:, ms], xtl0[:, cs], start=False, stop=False)
                    nc.tensor.matmul(ps[:], lhh1[:, ms], xtl1[:, cs], start=False, stop=False)
                    nc.tensor.matmul(ps[:], lhl0[:, ms], xth0[:, cs], start=False, stop=False)
                    nc.tensor.matmul(ps[:], lhl1[:, ms], xth1[:, cs], start=False, stop=False)
                    nc.tensor.matmul(ps[:], one_t[:2, :], msq_t[:2, cs], start=False, stop=True)
                    nc.vector.max(out=cand[:, nb * 8 : (nb + 1) * 8], in_=ps[:])
                    nc.vector.max_index(
                        out=relx[:, nb * 8 : (nb + 1) * 8],
                        in_max=cand[:, nb * 8 : (nb + 1) * 8],
                        in_values=ps[:],
                    )
                vals = work.tile([P, 16], f32, tag="vals")
                pos = work.tile([P, 16], u32, tag="pos")
                cmod = work.tile([P, CPC], f32, tag="cmod")
                nc.vector.max(out=vals[:, 0:8], in_=cand[:])
                nc.vector.max_index(out=pos[:, 0:8], in_max=vals[:, 0:8], in_values=cand[:])
                nc.vector.match_replace(
                    out=cmod[:], in_to_replace=vals[:, 0:8], in_values=cand[:], imm_value=NEG_BIG
                )
                nc.vector.max(out=vals[:, 8:16], in_=cmod[:])
                nc.vector.max_index(out=pos[:, 8:16], in_max=vals[:, 8:16], in_values=cmod[:])
                nc.sync.dma_start(pos_d.ap()[blk], pos[:])
                nc.sync.dma_start(rel_d.ap()[blk], relx[:])
                nc.sync.dma_start(val_d.ap()[blk], vals[:])
    return nc


def build_phase2() -> bass.Bass:
    nc = bass.Bass("TRN2", target_bir_lowering=False, debug=False)
    xf_d = nc.dram_tensor("xf", [N, D], f32, kind="ExternalInput")
    xl_d = nc.dram_tensor("xl", [ROWS, D], f32, kind="ExternalInput")
    nbr_d = nc.dram_tensor("nbr", [ROWS, KK], i32, kind="ExternalInput")
    nsc_d = nc.dram_tensor("nsc", [P, 1], f32, kind="ExternalInput")
    lp_d = nc.dram_tensor("lp", [BLKS, P, KK], f32, kind="ExternalOutput")

    with tile.TileContext(nc) as tc:
        with (
            tc.tile_pool(name="const", bufs=1) as const,
            tc.tile_pool(name="work", bufs=3) as work,
        ):
            nsc_t = const.tile([P, 1], f32)
            nc.sync.dma_start(nsc_t[:], nsc_d.ap())
            for blk in range(BLKS):
                rs = slice(blk * P, (blk + 1) * P)
                idx_t = work.tile([P, KK], i32, tag="idx")
                nc.sync.dma_start(idx_t[:], nbr_d.ap()[rs, :])
                xl_t = work.tile([P, D], f32, tag="xl")
                nc.sync.dma_start(xl_t[:], xl_d.ap()[rs, :])
                G = work.tile([P, KK, D], f32, tag="G")
                for c in range(KK):
                    nc.gpsimd.indirect_dma_start(
                        out=G[:, c, :],
                        out_offset=None,
                        in_=xf_d.ap(),
                        in_offset=bass.IndirectOffsetOnAxis(ap=idx_t[:, c : c + 1], axis=0),
                    )
                diff = work.tile([P, KK, D], f32, tag="diff")
                nc.vector.tensor_tensor(
                    out=diff[:],
                    in0=G[:],
                    in1=xl_t[:, None, :].to_broadcast([P, KK, D]),
                    op=mybir.AluOpType.subtract,
                )
                d2 = work.tile([P, KK, D], f32, tag="d2")
                nc.scalar.activation(
                    out=d2[:], in_=diff[:], func=mybir.ActivationFunctionType.Square
                )
                ss = work.tile([P, KK], f32, tag="ss")
                nc.vector.tensor_reduce(
                    out=ss[:], in_=d2[:], axis=mybir.AxisListType.X, op=mybir.AluOpType.add
                )
                lp_t = work.tile([P, KK], f32, tag="lp")
                nc.vector.tensor_scalar_mul(lp_t[:], ss[:], nsc_t[:, :1])
                nc.sync.dma_start(lp_d.ap()[blk], lp_t[:])
    return nc


_cache = {}


def _get_phase1():
    if "p1" not in _cache:
        _cache["p1"] = build_phase1()
    return _cache["p1"]


def _get_phase2():
    if "p2" not in _cache:
        _cache["p2"] = build_phase2()
    return _cache["p2"]


def _split_f16(a: np.ndarray):
    h = a.astype(np.float16)
    l = (a - h.astype(np.float32)).astype(np.float16)
    return np.ascontiguousarray(h), np.ascontiguousarray(l)


def phase1_inputs(x2: np.ndarray):
    xt = np.ascontiguousarray(x2.T)  # [D, N]
    sq = np.einsum("nd,nd->n", x2, x2, dtype=np.float32)
    msqv = (-0.5 * sq).astype(np.float32)
    xth, xtl = _split_f16(xt)
    mh, ml = _split_f16(msqv)
    msq2 = np.ascontiguousarray(np.stack([mh, ml], 0))  # [2, N]
    one2 = np.ones((2, P), np.float16)
    maps = []
    for c in range(CORES):
        lh = np.ascontiguousarray(xt[:, c * ROWS : (c + 1) * ROWS])
        lhh, lhl = _split_f16(lh)
        maps.append({"xth": xth, "xtl": xtl, "lhh": lhh, "lhl": lhl, "msq": msq2, "one": one2})
    return maps


def phase1_postprocess(results) -> np.ndarray:
    idx = np.empty((N, KK), np.int32)
    for c in range(CORES):
        pos = results[c]["pos"].reshape(ROWS, 16)[:, :KK].astype(np.int64)
        rel = results[c]["rel"].reshape(ROWS, CPC)
        chunk = pos >> 3
        relv = np.take_along_axis(rel, pos, axis=1).astype(np.int64)
        idx[c * ROWS : (c + 1) * ROWS] = (chunk * CH + relv).astype(np.int32)
    return idx


def make_edges(idx: np.ndarray):
    """Replicates the reference edge construction for b=1."""
    indices = idx.T[None]  # [1, k, n]
    flat = np.ascontiguousarray(indices.reshape(1, KK * N)).astype(np.int32)
    rows = np.tile(np.arange(N, dtype=np.int32)[None, :, None], (1, 1, KK)).reshape(1, -1)
    edges = np.stack((flat, rows), axis=-1)  # [1, n*k, 2]
    edges = edges.transpose(1, 0, 2).reshape(2, -1)
    return edges, flat


def phase2_inputs(x2: np.ndarray, flat: np.ndarray, scale: np.float32):
    aidx = np.ascontiguousarray(flat.reshape(N, KK)).astype(np.int32)
    nsc = np.full((P, 1), -scale, np.float32)
    maps = []
    for c in range(CORES):
        rs = slice(c * ROWS, (c + 1) * ROWS)
        maps.append(
            {
                "xf": x2,
                "xl": np.ascontiguousarray(x2[rs]),
                "nbr": np.ascontiguousarray(aidx[rs]),
                "nsc": nsc,
            }
        )
    return maps


def kernel(x, temperature, A=None):
    x = np.asarray(x)
    if x.ndim == 2:
        x = x[None]
    x = x.astype(np.float32, copy=False)
    x2 = np.ascontiguousarray(x.reshape(N, D))
    temp = np.asarray(temperature).astype(np.float32)
    scale = np.float32(np.exp(np.clip(temp[0], -5.0, 5.0)))

    nc1 = _get_phase1()
    res1 = run_bass_kernel_spmd(nc1, phase1_inputs(x2), core_ids=list(range(CORES)))
    idx = phase1_postprocess(res1.results)
    edges, flat = make_edges(idx)

    nc2 = _get_phase2()
    res2 = run_bass_kernel_spmd(nc2, phase2_inputs(x2, flat, scale), core_ids=list(range(CORES)))
    lp = np.concatenate(
        [res2.results[c]["lp"].reshape(ROWS, KK) for c in range(CORES)], axis=0
    )[None]
    return x, edges, lp


# revision 4
# speedup vs baseline: 1.0751x; 1.0751x over previous
"""KNN-graph (DGM euclidean) kernel for Trainium2, sharded over 8 NeuronCores.

Problem: x [1, 8192, 256] f32 -> (x, edges [2, 81920] i32, logprobs [1, 8192, 10] f32)
(k=10 nearest neighbors under squared euclidean distance, scaled by
exp(clip(temperature, -5, 5)); edges/logprobs replicate the torch DGM layout
quirks of the reference, including the k*n vs n*k pairing mismatch in x1/x2.)

Sharding: rows are split 1024-per-core (8 cores); every core receives the
full x^T for the rhs of its [1024, 8192] distance-tile matmul (host-side
"all-gather").

Per core:
  phase 1 (top-k): neg_rank[i,j] = <x_i,x_j> - sq_j/2 is monotone-decreasing
    in ||x_i - x_j||^2, so top-10-smallest-distance == top-10-largest
    neg_rank.  The gram matrix is computed in fp16 hi/lo split form
    (x = h + l exactly, g = h.h + h.l + l.h + l.l accumulated in fp32 PSUM
    -> fp32-grade precision at the PE's full fp16 rate; plain fp32 matmuls
    run at 1/4 rate).  The -sq_j/2 term rides in as one K=2 matmul against a
    constant ones lhsT.  Distances land in [128, 512] PSUM chunks; the DVE's
    max (top-8 per partition) + max_index (their within-chunk indices) reduce
    each chunk; a second max/max_index/match_replace/max/max_index pass over
    the 128 per-row candidates yields the global top-16 values + candidate
    positions.  Host maps (position -> chunk, within-chunk index) to absolute
    neighbor indices.  Exact-duplicate values are handled the same way
    jax.lax.top_k does (ascending index for ties): max_index assigns
    successive occurrences to successive ranks, and ranks 9-16 are searched
    in the match_replace'd candidate array so values spanning the 8/9
    boundary resolve to the correct occurrence.
  phase 2 (logprobs): the reference pairs x1[m]=x[flat[m]] (order k*n) with
    x2[m]=x[m//k] (order n*k), i.e. logprobs[i,c] = -scale*||x[A[i,c]]-x[i]||^2
    with A = flat.reshape(n, k), flat = idx.T.flatten().  A depends on the
    full idx, so it runs as a second launch: indirect-DMA row gather of
    x[A[i,c]], subtract broadcast x_i, Square on the scalar engine, free-dim
    reduce, scale by -scale.

Everything else (edge assembly, index arithmetic) is cheap integer
bookkeeping done on host, replicating the reference expressions verbatim.
"""

import numpy as np
import orjson

import concourse.bass as bass
import concourse.mybir as mybir
import concourse.tile as tile

# ---------------------------------------------------------------------------
# Toolchain workaround: the walrus build in this container rejects
# instructions carrying more than one sem-wait ("Too many sync wait
# commands", CoreV3GenImpl.cpp setupSyncWait).  TileContext emits multi-wait
# instructions (notably its tail drain, matmuls and DMA triggers).  Splitting
# is safe: extra waits are hoisted onto Drain instructions inserted
# immediately before the owner on the same engine queue — queues execute
# in-order, so all waits still complete before the instruction issues.
# ---------------------------------------------------------------------------
_MAX_WAITS = 1
_patched = False


def _split_bir_waits(bir_json: bytes) -> bytes:
    bir = orjson.loads(bir_json)
    n_fixed = 0
    for fn in bir.get("functions", []):
        for bb in fn.get("blocks", []):
            out = []
            for ins in bb.get("instructions", []):
                si = ins.get("sync_info")
                waits = si.get("on_wait") if si else None
                if waits and len(waits) > _MAX_WAITS:
                    extra = waits[_MAX_WAITS:]
                    si["on_wait"] = waits[:_MAX_WAITS]
                    for i in range(0, len(extra), _MAX_WAITS):
                        out.append(
                            {
                                "name": f"{ins['name']}-wsplit{i}",
                                "opcode": "Drain",
                                "engine": ins["engine"],
                                "ins": [],
                                "outs": [],
                                "debug": ins.get("debug", 0),
                                "sync_info": {
                                    "on_wait": extra[i : i + _MAX_WAITS],
                                    "on_update": [],
                                },
                            }
                        )
                        n_fixed += 1
                out.append(ins)
            bb["instructions"] = out
    if n_fixed:
        return orjson.dumps(bir)
    return bir_json


def install_patch():
    global _patched
    if _patched:
        return
    _patched = True
    from concourse import bass_utils, bass2jax

    orig = bass_utils.compile_bir_kernel

    def patched_compile_bir_kernel(bir_json, tmpdir, neff_name="file.neff"):
        return orig(_split_bir_waits(bytes(bir_json)), tmpdir, neff_name)

    bass_utils.compile_bir_kernel = patched_compile_bir_kernel
    bass2jax.compile_bir_kernel = patched_compile_bir_kernel


install_patch()

from concourse.bass_utils import run_bass_kernel_spmd  # noqa: E402

f32 = mybir.dt.float32
f16 = mybir.dt.float16
u32 = mybir.dt.uint32
i32 = mybir.dt.int32

N, D, KK = 8192, 256, 10
CORES = 8
ROWS = N // CORES  # 1024 rows per core
P = 128
BLKS = ROWS // P  # 8 row-blocks per core
CH = 512  # distance-chunk width (one PSUM bank)
NB = N // CH  # 16 chunks across the 8192 columns
CPC = 8 * NB  # candidates per row (128)
NEG_BIG = -3.0e38


def build_phase1() -> bass.Bass:
    nc = bass.Bass("TRN2", target_bir_lowering=False, debug=False)
    xth_d = nc.dram_tensor("xth", [D, N], f16, kind="ExternalInput")
    xtl_d = nc.dram_tensor("xtl", [D, N], f16, kind="ExternalInput")
    lhh_d = nc.dram_tensor("lhh", [D, ROWS], f16, kind="ExternalInput")
    lhl_d = nc.dram_tensor("lhl", [D, ROWS], f16, kind="ExternalInput")
    msq_d = nc.dram_tensor("msq", [2, N], f16, kind="ExternalInput")
    one_d = nc.dram_tensor("one", [2, P], f16, kind="ExternalInput")
    pos_d = nc.dram_tensor("pos", [BLKS, P, 16], u32, kind="ExternalOutput")
    rel_d = nc.dram_tensor("rel", [BLKS, P, CPC], u32, kind="ExternalOutput")
    val_d = nc.dram_tensor("val", [BLKS, P, 16], f32, kind="ExternalOutput")

    with tile.TileContext(nc) as tc:
        with (
            tc.tile_pool(name="const", bufs=1) as const,
            tc.tile_pool(name="work", bufs=3) as work,
            tc.tile_pool(name="psp", bufs=8, space="PSUM") as psum,
        ):
            # Small lhsT/msq constants first, then the big x^T tiles in
            # column chunks so block-0's matmuls start as soon as their
            # columns land instead of waiting ~28us for the full 10MB load.
            lhh0 = const.tile([P, ROWS], f16)
            lhh1 = const.tile([P, ROWS], f16)
            lhl0 = const.tile([P, ROWS], f16)
            lhl1 = const.tile([P, ROWS], f16)
            msq_t = const.tile([2, N], f16)
            one_t = const.tile([2, P], f16)
            nc.sync.dma_start(one_t[:], one_d.ap())
            nc.sync.dma_start(lhh0[:], lhh_d.ap()[0:P, :])
            nc.sync.dma_start(lhh1[:], lhh_d.ap()[P : 2 * P, :])
            nc.sync.dma_start(lhl0[:], lhl_d.ap()[0:P, :])
            nc.sync.dma_start(lhl1[:], lhl_d.ap()[P : 2 * P, :])
            nc.sync.dma_start(msq_t[:], msq_d.ap())
            xth0 = const.tile([P, N], f16)
            xth1 = const.tile([P, N], f16)
            xtl0 = const.tile([P, N], f16)
            xtl1 = const.tile([P, N], f16)
            LD_CHUNKS = 8
            W = N // LD_CHUNKS
            for lc in range(LD_CHUNKS):
                s = slice(lc * W, (lc + 1) * W)
                nc.sync.dma_start(xth0[:, s], xth_d.ap()[0:P, s])
                nc.sync.dma_start(xth1[:, s], xth_d.ap()[P : 2 * P, s])
                nc.sync.dma_start(xtl0[:, s], xtl_d.ap()[0:P, s])
                nc.sync.dma_start(xtl1[:, s], xtl_d.ap()[P : 2 * P, s])

            for blk in range(BLKS):
                cand = work.tile([P, CPC], f32, tag="cand")
                relx = work.tile([P, CPC], u32, tag="relx")
                ms = slice(blk * P, (blk + 1) * P)
                for nb in range(NB):
                    cs = slice(nb * CH, (nb + 1) * CH)
                    ps = psum.tile([P, CH], f32, tag="ps")
                    # g = h.h + h.l + l.h + l.l  (exact fp32-grade gram), plus
                    # the -sq_j/2 row via a K=2 ones matmul.
                    nc.tensor.matmul(ps[:], lhh0[:, ms], xth0[:, cs], start=True, stop=False)
                    nc.tensor.matmul(ps[:], lhh1[:, ms], xth1[:, cs], start=False, stop=False)
                    nc.tensor.matmul(ps[:], lhh0[:, ms], xtl0[:, cs], start=False, stop=False)
                    nc.tensor.matmul(ps[:], lhh1[:, ms], xtl1[:, cs], start=False, stop=False)
                    nc.tensor.matmul(ps[:], lhl0[:, ms], xth0[:, cs], start=False, stop=False)
                    nc.tensor.matmul(ps[:], lhl1[:, ms], xth1[:, cs], start=False, stop=False)
                    nc.tensor.matmul(ps[:], one_t[:2, :], msq_t[:2, cs], start=False, stop=True)
                    nc.vector.max(out=cand[:, nb * 8 : (nb + 1) * 8], in_=ps[:])
                    nc.vector.max_index(
                        out=relx[:, nb * 8 : (nb + 1) * 8],
                        in_max=cand[:, nb * 8 : (nb + 1) * 8],
                        in_values=ps[:],
                    )
                vals = work.tile([P, 16], f32, tag="vals")
                pos = work.tile([P, 16], u32, tag="pos")
                cmod = work.tile([P, CPC], f32, tag="cmod")
                nc.vector.max(out=vals[:, 0:8], in_=cand[:])
                nc.vector.max_index(out=pos[:, 0:8], in_max=vals[:, 0:8], in_values=cand[:])
                nc.vector.match_replace(
                    out=cmod[:], in_to_replace=vals[:, 0:8], in_values=cand[:], imm_value=NEG_BIG
                )
                nc.vector.max(out=vals[:, 8:16], in_=cmod[:])
                nc.vector.max_index(out=pos[:, 8:16], in_max=vals[:, 8:16], in_values=cmod[:])
                nc.sync.dma_start(pos_d.ap()[blk], pos[:])
                nc.sync.dma_start(rel_d.ap()[blk], relx[:])
                nc.sync.dma_start(val_d.ap()[blk], vals[:])
    return nc


def build_phase2() -> bass.Bass:
    nc = bass.Bass("TRN2", target_bir_lowering=False, debug=False)
    xf_d = nc.dram_tensor("xf", [N, D], f32, kind="ExternalInput")
    xl_d = nc.dram_tensor("xl", [ROWS, D], f32, kind="ExternalInput")
    nbr_d = nc.dram_tensor("nbr", [ROWS, KK], i32, kind="ExternalInput")
    nsc_d = nc.dram_tensor("nsc", [P, 1], f32, kind="ExternalInput")
    lp_d = nc.dram_tensor("lp", [BLKS, P, KK], f32, kind="ExternalOutput")

    with tile.TileContext(nc) as tc:
        with (
            tc.tile_pool(name="const", bufs=1) as const,
            tc.tile_pool(name="work", bufs=3) as work,
        ):
            nsc_t = const.tile([P, 1], f32)
            nc.sync.dma_start(nsc_t[:], nsc_d.ap())
            for blk in range(BLKS):
                rs = slice(blk * P, (blk + 1) * P)
                idx_t = work.tile([P, KK], i32, tag="idx")
                nc.sync.dma_start(idx_t[:], nbr_d.ap()[rs, :])
                xl_t = work.tile([P, D], f32, tag="xl")
                nc.sync.dma_start(xl_t[:], xl_d.ap()[rs, :])
                G = work.tile([P, KK, D], f32, tag="G")
                # one indirect row-gather per neighbor slot; a single batched
                # [128, 10]-offset gather models 40us faster but returns wrong
                # data on real hardware (CoreSim accepts it), so keep 10
                # single-column gathers.
                for c in range(KK):
                    nc.gpsimd.indirect_dma_start(
                        out=G[:, c, :],
                        out_offset=None,
                        in_=xf_d.ap(),
                        in_offset=bass.IndirectOffsetOnAxis(ap=idx_t[:, c : c + 1], axis=0),
                    )
                diff = work.tile([P, KK, D], f32, tag="diff")
                nc.vector.tensor_tensor(
                    out=diff[:],
                    in0=G[:],
                    in1=xl_t[:, None, :].to_broadcast([P, KK, D]),
                    op=mybir.AluOpType.subtract,
                )
                d2 = work.tile([P, KK, D], f32, tag="d2")
                nc.scalar.activation(
                    out=d2[:], in_=diff[:], func=mybir.ActivationFunctionType.Square
                )
                ss = work.tile([P, KK], f32, tag="ss")
                nc.vector.tensor_reduce(
                    out=ss[:], in_=d2[:], axis=mybir.AxisListType.X, op=mybir.AluOpType.add
                )
                lp_t = work.tile([P, KK], f32, tag="lp")
                nc.vector.tensor_scalar_mul(lp_t[:], ss[:], nsc_t[:, :1])
                nc.sync.dma_start(lp_d.ap()[blk], lp_t[:])
    return nc


_cache = {}


def _get_phase1():
    if "p1" not in _cache:
        _cache["p1"] = build_phase1()
    return _cache["p1"]


def _get_phase2():
    if "p2" not in _cache:
        _cache["p2"] = build_phase2()
    return _cache["p2"]


def _split_f16(a: np.ndarray):
    h = a.astype(np.float16)
    l = (a - h.astype(np.float32)).astype(np.float16)
    return np.ascontiguousarray(h), np.ascontiguousarray(l)


def phase1_inputs(x2: np.ndarray):
    xt = np.ascontiguousarray(x2.T)  # [D, N]
    sq = np.einsum("nd,nd->n", x2, x2, dtype=np.float32)
    msqv = (-0.5 * sq).astype(np.float32)
    xth, xtl = _split_f16(xt)
    mh, ml = _split_f16(msqv)
    msq2 = np.ascontiguousarray(np.stack([mh, ml], 0))  # [2, N]
    one2 = np.ones((2, P), np.float16)
    maps = []
    for c in range(CORES):
        lh = np.ascontiguousarray(xt[:, c * ROWS : (c + 1) * ROWS])
        lhh, lhl = _split_f16(lh)
        maps.append({"xth": xth, "xtl": xtl, "lhh": lhh, "lhl": lhl, "msq": msq2, "one": one2})
    return maps


def phase1_postprocess(results) -> np.ndarray:
    idx = np.empty((N, KK), np.int32)
    for c in range(CORES):
        pos = results[c]["pos"].reshape(ROWS, 16)[:, :KK].astype(np.int64)
        rel = results[c]["rel"].reshape(ROWS, CPC)
        chunk = pos >> 3
        relv = np.take_along_axis(rel, pos, axis=1).astype(np.int64)
        idx[c * ROWS : (c + 1) * ROWS] = (chunk * CH + relv).astype(np.int32)
    return idx


def make_edges(idx: np.ndarray):
    """Replicates the reference edge construction for b=1."""
    indices = idx.T[None]  # [1, k, n]
    flat = np.ascontiguousarray(indices.reshape(1, KK * N)).astype(np.int32)
    rows = np.tile(np.arange(N, dtype=np.int32)[None, :, None], (1, 1, KK)).reshape(1, -1)
    edges = np.stack((flat, rows), axis=-1)  # [1, n*k, 2]
    edges = edges.transpose(1, 0, 2).reshape(2, -1)
    return edges, flat


def phase2_inputs(x2: np.ndarray, flat: np.ndarray, scale: np.float32):
    aidx = np.ascontiguousarray(flat.reshape(N, KK)).astype(np.int32)
    nsc = np.full((P, 1), -scale, np.float32)
    maps = []
    for c in range(CORES):
        rs = slice(c * ROWS, (c + 1) * ROWS)
        maps.append(
            {
                "xf": x2,
                "xl": np.ascontiguousarray(x2[rs]),
                "nbr": np.ascontiguousarray(aidx[rs]),
                "nsc": nsc,
            }
        )
    return maps


def kernel(x, temperature, A=None):
    x = np.asarray(x)
    if x.ndim == 2:
        x = x[None]
    x = x.astype(np.float32, copy=False)
    x2 = np.ascontiguousarray(x.reshape(N, D))
    temp = np.asarray(temperature).astype(np.float32)
    scale = np.float32(np.exp(np.clip(temp[0], -5.0, 5.0)))

    nc1 = _get_phase1()
    res1 = run_bass_kernel_spmd(nc1, phase1_inputs(x2), core_ids=list(range(CORES)))
    idx = phase1_postprocess(res1.results)
    edges, flat = make_edges(idx)

    nc2 = _get_phase2()
    res2 = run_bass_kernel_spmd(nc2, phase2_inputs(x2, flat, scale), core_ids=list(range(CORES)))
    lp = np.concatenate(
        [res2.results[c]["lp"].reshape(ROWS, KK) for c in range(CORES)], axis=0
    )[None]
    return x, edges, lp
